# revision 1
# baseline (speedup 1.0000x reference)
"""Multi-head attention (dense_transformer) Trainium2 Bass kernel.

Problem: x[8, 512, 32, 32]; per-batch 1x1-conv QKV projections, 8-head
attention over N=H*W=1024 positions (head_dim 64), output projection,
residual. Sharding: data-parallel over batch B=8 across the 8 cores —
one batch element per core, no collectives.

Per-core dataflow (all matmul inputs bf16, accumulation fp32):
  - Host pre-transposes weights to [c, o] layout and pre-casts to bf16.
  - Q, K in [c, n] layout: Q[ot] = WqT[ct].T @ x16[ct] (+bq).
  - V kept transposed: VT[jt][n, o] = x16[:, jt].T @ WvT (+bv), stored
    per-head with a ones column appended: [128, 8 heads, 65].
  - S^T[j, i] = K_h.T Q_h per head: j on partitions -> AV matmul needs
    no transposes anywhere. exp via ScalarE with the 1/sqrt(64) scale
    folded in; softmax denominator comes from the VT ones column during
    the AV matmul (PSUM row 64); normalization = reciprocal + DRAM-
    bounce partition broadcast + VectorE multiply.
  - out = WoT.T @ O + (x32 + bo prefolded), DMA'd out in fp32.

PSUM (8 banks) is phase-scoped: projections use a 4-buf half-bank pool
that closes before the AV-accumulator pool opens in the same banks.
"""

import sys

if "/opt/trn_rl_repo" not in sys.path:
    sys.path.insert(0, "/opt/trn_rl_repo")

import numpy as np
import ml_dtypes

import concourse.bass as bass
import concourse.mybir as mybir
from concourse.tile import TileContext

DIM = 512
NH = 8
HD = 64
N = 1024
P = 128
CT = DIM // P  # 4 c-tiles of 128 channels
JT = N // P    # 8 j-tiles of 128 positions
F32 = mybir.dt.float32
BF16 = mybir.dt.bfloat16
AOP = mybir.AluOpType
EXP = mybir.ActivationFunctionType.Exp


class FixedTileContext(TileContext):
    """Works around a walrus/bass snapshot mismatch: this walrus build
    accepts only one sync-wait command per instruction, but Tile's wait
    assigner happily attaches several. After scheduling, excess waits on
    any instruction are peeled off onto same-engine NOPs inserted right
    before it (same blocking semantics: the engine executes in order)."""

    MAX_WAITS = 1
    MAX_WAITS_DATA = 1
    _wsplit_ctr = 0

    def _split_sync_waits(self):
        seq_only = mybir.SEQUENCER_ONLY_OPCODES
        for fn in self.nc.m.functions:
            for blk in fn.blocks:
                insts = list(blk.instructions)
                out = []
                for inst in insts:
                    si = inst.sync_info
                    limit = (
                        self.MAX_WAITS
                        if inst.opcode in seq_only
                        else self.MAX_WAITS_DATA
                    )
                    if si is not None and len(si.on_wait) > limit:
                        waits = list(si.on_wait)
                        movers = waits[:-limit]
                        keep = waits[-limit:]
                        del si.on_wait[:]
                        for w in keep:
                            si.on_wait.append(w)
                        for w in movers:
                            FixedTileContext._wsplit_ctr += 1
                            nop = mybir.InstNoOp(
                                name=f"wsplit-{FixedTileContext._wsplit_ctr}",
                                ins=[],
                                outs=[],
                            )
                            nop.engine = inst.engine
                            nop.sync_info = mybir.SyncInfo(on_wait=[w], on_update=[])
                            out.append(nop)
                    out.append(inst)
                if len(out) != len(insts):
                    del blk.instructions[:]
                    for i in out:
                        blk.add_instruction(i)

    split_on_exit = True

    def __exit__(self, *exc):
        ret = super().__exit__(*exc)
        if exc[0] is None and self.split_on_exit:
            self._split_sync_waits()
        return ret


def build_nc(split_waits=True):
    nc = bass.Bass()

    x32d = nc.dram_tensor("x32", [DIM, N], F32, kind="ExternalInput")
    x16d = nc.dram_tensor("x16", [DIM, N], BF16, kind="ExternalInput")
    wqd = nc.dram_tensor("wqt", [DIM, DIM], BF16, kind="ExternalInput")
    wkd = nc.dram_tensor("wkt", [DIM, DIM], BF16, kind="ExternalInput")
    wvd = nc.dram_tensor("wvt", [DIM, DIM], BF16, kind="ExternalInput")
    wod = nc.dram_tensor("wot", [DIM, DIM], BF16, kind="ExternalInput")
    bqd = nc.dram_tensor("bq", [DIM], F32, kind="ExternalInput")
    bkd = nc.dram_tensor("bk", [DIM], F32, kind="ExternalInput")
    bvd = nc.dram_tensor("bv", [DIM], F32, kind="ExternalInput")
    bod = nc.dram_tensor("bo", [DIM], F32, kind="ExternalInput")
    outd = nc.dram_tensor("out", [DIM, N], F32, kind="ExternalOutput")

    FixedTileContext.split_on_exit = split_waits
    with FixedTileContext(nc) as tc:
        with (
            tc.tile_pool(name="persist", bufs=1) as persist,
            tc.tile_pool(name="ppool", bufs=32) as ppool,
            tc.tile_pool(name="small", bufs=3) as small,
            tc.tile_pool(name="otile", bufs=8) as otile,
            tc.tile_pool(name="dram", bufs=1, space="DRAM") as dram,
            tc.tile_pool(name="psS", bufs=2, space="PSUM") as psS_pool,
        ):
            # weights/biases ride ScalarE's DMA queues (ScalarE is idle
            # until the first exp) so they don't serialize behind the x
            # loads on SP's queues
            def load_w(wd, name):
                wr = wd.rearrange("(t p) o -> t p o", p=P)
                ws = []
                for t in range(CT):
                    wt = persist.tile(
                        [P, DIM], BF16, tag=f"{name}_{t}", name=f"{name}_{t}"
                    )
                    nc.scalar.dma_start(out=wt, in_=wr[t])
                    ws.append(wt)
                return ws

            def load_b(bd, name):
                bt = persist.tile([P, CT], F32, tag=name, name=name)
                nc.scalar.dma_start(out=bt, in_=bd.rearrange("(t p) -> p t", p=P))
                return bt

            # S^T + exp for one head pair. Emission alternates PE row
            # groups 0-63 / 64-127 between consecutive matmuls so the
            # hardware overlaps them (per-subarray concurrency) even
            # though K=64 only half-fills the array.
            def s_phase(pair):
                P16 = {}
                for jt in range(JT):
                    tiles = {}

                    def smm(h2, ih):
                        base = 64 * h2
                        nc.tensor.matmul(
                            tiles[h2][:, ih * 512 : (ih + 1) * 512],
                            lhsT=K[pair][base : base + 64, jt * P : (jt + 1) * P],
                            rhs=Q[pair][base : base + 64, ih * 512 : (ih + 1) * 512],
                            start=True,
                            stop=True,
                        )

                    tiles[0] = psS_pool.tile([P, N], F32, tag="psS", name="psS")
                    smm(0, 0)
                    tiles[1] = psS_pool.tile([P, N], F32, tag="psS", name="psS")
                    smm(1, 0)
                    smm(0, 1)
                    smm(1, 1)
                    for h2 in range(2):
                        pt = ppool.tile([P, N], BF16, tag="p16", name="p16")
                        nc.scalar.activation(pt, tiles[h2], EXP, scale=0.125)
                        P16[(jt, h2)] = pt
                return P16

            def p16_slice(P16, jt, h2, ih):
                return P16[(jt, h2)][:, ih * 512 : (ih + 1) * 512]

            # AV matmul + softmax normalization for one head pair. The raw
            # head output is copied out of PSUM right away (frees the psO
            # slot for the next head's AV); the DRAM-bounce broadcast and
            # the normalize multiply then run off the critical PSUM path.
            def av_phase(pair, P16, psO_pool, O16, rdram):
                last_pair = pair == NH // 2 - 1
                h2_order = (1, 0) if last_pair else (0, 1)
                for h2 in h2_order:
                    h = 2 * pair + h2
                    rec = small.tile([HD + 1, N], F32, tag="rec", name="rec")
                    oraw = small.tile([HD, N], F32, tag="oraw", name="oraw")
                    rb = small.tile([HD, N], F32, tag="rb", name="rb")
                    for ih in range(2):
                        sl = slice(ih * 512, (ih + 1) * 512)
                        po = psO_pool.tile([HD + 1, 512], F32, tag="psO", name="po")
                        for jt in range(JT):
                            nc.tensor.matmul(
                                po,
                                lhsT=VT[jt][:, h, :],
                                rhs=p16_slice(P16, jt, h2, ih),
                                start=(jt == 0),
                                stop=(jt == JT - 1),
                            )
                        # softmax denominator sits in row HD of po
                        nc.vector.reciprocal(rec[HD : HD + 1, sl], po[HD : HD + 1, :])
                        # copy the raw head output out of PSUM immediately
                        # (frees the psO slot); on the last pair ScalarE is
                        # done with exps, so use it and keep DVE off the
                        # critical chain
                        if last_pair:
                            nc.scalar.copy(oraw[:, sl], po[0:HD, :])
                        else:
                            nc.vector.tensor_copy(oraw[:, sl], po[0:HD, :])
                        # per-half DRAM bounce broadcasts 1/colsum across
                        # partitions (SBUF APs reject 0 partition stride)
                        dmae = nc.scalar if last_pair else nc.sync
                        dmae.dma_start(
                            out=rdram[h : h + 1, sl], in_=rec[HD : HD + 1, sl]
                        )
                        rsrc = rdram[h : h + 1, sl]
                        nc.sync.dma_start(
                            out=rb[:, sl],
                            in_=bass.AP(
                                tensor=rsrc.tensor,
                                offset=rsrc.offset,
                                ap=[[0, HD]] + list(rsrc.ap[1:]),
                            ),
                        )
                    osc = None
                    if h2 != 0:
                        osc = small.tile([HD, N], BF16, tag="osc", name="osc")
                    for ih in range(2):
                        sl = slice(ih * 512, (ih + 1) * 512)
                        if h2 == 0:
                            nc.vector.tensor_tensor(
                                O16[pair][0:HD, sl], oraw[:, sl], rb[:, sl], AOP.mult
                            )
                        else:
                            nc.vector.tensor_tensor(
                                osc[:, sl], oraw[:, sl], rb[:, sl], AOP.mult
                            )
                            (nc.scalar if last_pair else nc.sync).dma_start(
                                out=O16[pair][HD:P, sl], in_=osc[:, sl]
                            )

            with tc.tile_pool(name="pp", bufs=4, space="PSUM") as pp:
                # ---------- input loads ----------
                x16r = x16d.rearrange("(t p) n -> t p n", p=P)
                xs16 = []
                for t in range(CT):
                    xt = persist.tile([P, N], BF16, tag=f"x16_{t}", name=f"x16_{t}")
                    nc.sync.dma_start(out=xt, in_=x16r[t])
                    xs16.append(xt)

                # interleave wq/wk tiles so K0's accumulation matmuls can
                # trickle-start alongside Q0's instead of waiting for the
                # whole of wq to finish on the same queue
                wqr = wqd.rearrange("(t p) o -> t p o", p=P)
                wkr = wkd.rearrange("(t p) o -> t p o", p=P)
                wqs, wks = [], []
                for t in range(CT):
                    wqt_ = persist.tile([P, DIM], BF16, tag=f"wq_{t}", name=f"wq_{t}")
                    nc.scalar.dma_start(out=wqt_, in_=wqr[t])
                    wqs.append(wqt_)
                    wkt_ = persist.tile([P, DIM], BF16, tag=f"wk_{t}", name=f"wk_{t}")
                    nc.scalar.dma_start(out=wkt_, in_=wkr[t])
                    wks.append(wkt_)
                bq_sb = load_b(bqd, "bq")
                bk_sb = load_b(bkd, "bk")

                # trigger the ~2.7us exp table load on ScalarE right after
                # its weight-DMA issues, so the first real exp doesn't pay it
                warm = small.tile([1, 8], F32, tag="warm", name="warm")
                nc.vector.memset(warm, 0.0)
                nc.scalar.activation(warm, warm, EXP)

                # ------ Q, K projections: [CT][128, N] bf16, [c, n] layout
                def project_one(ws, b_sb, name, ot):
                    qt = persist.tile(
                        [P, N], BF16, tag=f"{name}_{ot}", name=f"{name}_{ot}"
                    )
                    for nh in range(2):
                        ps = pp.tile(
                            [P, 512], F32, tag="pp", name=f"pp_{name}{ot}{nh}"
                        )
                        for ct in range(CT):
                            nc.tensor.matmul(
                                ps,
                                lhsT=ws[ct][:, ot * P : (ot + 1) * P],
                                rhs=xs16[ct][:, nh * 512 : (nh + 1) * 512],
                                start=(ct == 0),
                                stop=(ct == CT - 1),
                            )
                        nc.vector.tensor_scalar_add(
                            qt[:, nh * 512 : (nh + 1) * 512],
                            ps,
                            b_sb[:, ot : ot + 1],
                        )
                    return qt

                Q, K = [], []
                Q.append(project_one(wqs, bq_sb, "q", 0))
                K.append(project_one(wks, bk_sb, "k", 0))

                # pair 0's S^T + exp right away: gets ScalarE going while
                # the remaining projections stream on the PE
                P16_0 = s_phase(0)
                Q.append(project_one(wqs, bq_sb, "q", 1))
                K.append(project_one(wks, bk_sb, "k", 1))
                P16_1 = s_phase(1)

                # ------ V^T projection: VT[jt] = [128, NH, HD+1] bf16
                wvs = load_w(wvd, "wv")
                bvB = persist.tile([P, DIM], F32, tag="bvB", name="bvB")
                nc.gpsimd.dma_start(
                    out=bvB,
                    in_=bass.AP(
                        tensor=bvd[:].tensor, offset=0, ap=[[0, P], [1, DIM]]
                    ),
                )
                VT = []
                for jt in range(JT):
                    vt = persist.tile(
                        [P, NH, HD + 1], BF16, tag=f"vt_{jt}", name=f"vt_{jt}"
                    )
                    ps = pp.tile([P, 512], F32, tag="pp", name=f"pp_v{jt}")
                    for ct in range(CT):
                        nc.tensor.matmul(
                            ps,
                            lhsT=xs16[ct][:, jt * P : (jt + 1) * P],
                            rhs=wvs[ct],
                            start=(ct == 0),
                            stop=(ct == CT - 1),
                        )
                    nc.vector.tensor_tensor(
                        vt[:, :, 0:HD],
                        ps.rearrange("p (h d) -> p h d", h=NH),
                        bvB.rearrange("p (h d) -> p h d", h=NH),
                        AOP.add,
                    )
                    nc.vector.memset(vt[:, :, HD : HD + 1], 1.0)
                    VT.append(vt)

                for ot in range(2, CT):
                    Q.append(project_one(wqs, bq_sb, "q", ot))
                    K.append(project_one(wks, bk_sb, "k", ot))

            # ---------- attention (heads 2p / 2p+1 live on partitions
            # 0-63 / 64-127 of Q/K c-tile p); the AV-accumulator pool
            # reuses banks the projection pool just released
            O16 = [
                persist.tile([P, N], BF16, tag=f"o16_{t}", name=f"o16_{t}")
                for t in range(CT)
            ]
            rdram = dram.tile([NH, N], F32, tag="rdram", name="rdram")
            with tc.tile_pool(name="psO", bufs=4, space="PSUM") as psO_pool:
                av_phase(0, P16_0, psO_pool, O16, rdram)
                P16_2 = s_phase(2)
                av_phase(1, P16_1, psO_pool, O16, rdram)
                P16_3 = s_phase(3)
                av_phase(2, P16_2, psO_pool, O16, rdram)
                av_phase(3, P16_3, psO_pool, O16, rdram)

                # loads for the output projection (low priority; the DMA
                # queues have slack mid-kernel)
                wos = load_w(wod, "wo")
                bo_sb = load_b(bod, "bo")
                x32r = x32d.rearrange("(t p) n -> t p n", p=P)
                xs32 = []
                for t in range(CT):
                    xt32 = persist.tile(
                        [P, N], F32, tag=f"x32_{t}", name=f"x32_{t}"
                    )
                    nc.sync.dma_start(out=xt32, in_=x32r[t])
                    nc.vector.tensor_scalar_add(xt32, xt32, bo_sb[:, t : t + 1])
                    xs32.append(xt32)

            # ---------- output projection + residual. ot0/ot1 psum tiles
            # come from the psS pool (slots drained by pair-3 exps);
            # ot2/ot3 from a pool reusing the psO banks (drained by the
            # early PSUM copies) — all 24 ct0-2 matmuls can therefore run
            # while the last head's epilogue is still in flight.
            with tc.tile_pool(name="po3", bufs=2, space="PSUM") as po3:
                outr = outd.rearrange("(t p) n -> t p n", p=P)

                def op_pre(ot, pool=None):
                    # ct 0..2 accumulation: issuable while the last head
                    # pair (feeding O16[3]) is still in its epilogue
                    if pool is None:
                        ps = psS_pool.tile([P, N], F32, tag="psS", name=f"ps_o{ot}")
                    else:
                        ps = pool.tile([P, N], F32, tag="op34", name=f"ps_o{ot}")
                    for nh in range(2):
                        for ct in range(CT - 1):
                            nc.tensor.matmul(
                                ps[:, nh * 512 : (nh + 1) * 512],
                                lhsT=wos[ct][:, ot * P : (ot + 1) * P],
                                rhs=O16[ct][:, nh * 512 : (nh + 1) * 512],
                                start=(ct == 0),
                                stop=(ct == CT - 2),
                            )
                    return ps

                def op_post(ot, ps):
                    # ct 3 continues the accumulation in a second group,
                    # then bias+residual and writeback
                    for nh in range(2):
                        nc.tensor.matmul(
                            ps[:, nh * 512 : (nh + 1) * 512],
                            lhsT=wos[CT - 1][:, ot * P : (ot + 1) * P],
                            rhs=O16[CT - 1][:, nh * 512 : (nh + 1) * 512],
                            start=False,
                            stop=True,
                            skip_group_check=True,
                        )
                    for nh in range(2):
                        ob = otile.tile([P, 512], F32, tag="ob", name="ob")
                        nc.vector.tensor_tensor(
                            ob,
                            ps[:, nh * 512 : (nh + 1) * 512],
                            xs32[ot][:, nh * 512 : (nh + 1) * 512],
                            AOP.add,
                        )
                        nc.sync.dma_start(
                            out=outr[ot][:, nh * 512 : (nh + 1) * 512], in_=ob
                        )

                ps0 = op_pre(0)
                ps1 = op_pre(1)
                ps2 = op_pre(2, po3)
                ps3 = op_pre(3, po3)
                op_post(0, ps0)
                op_post(1, ps1)
                op_post(2, ps2)
                op_post(3, ps3)
    return nc


_BF = ml_dtypes.bfloat16


def _prep_maps(x, Wq, bq, Wk, bk, Wv, bv, Wo, bo):
    # plain numpy up front: inputs may arrive as jax device arrays and
    # transforming those would trigger on-device jax execution
    x, Wq, bq, Wk, bk, Wv, bv, Wo, bo = (
        np.asarray(a) for a in (x, Wq, bq, Wk, bk, Wv, bv, Wo, bo)
    )
    B, C, H, W = x.shape
    xf = np.ascontiguousarray(x.reshape(B, C, H * W)).astype(np.float32)
    shared = {
        "wqt": np.ascontiguousarray(Wq.T).astype(_BF),
        "wkt": np.ascontiguousarray(Wk.T).astype(_BF),
        "wvt": np.ascontiguousarray(Wv.T).astype(_BF),
        "wot": np.ascontiguousarray(Wo.T).astype(_BF),
        "bq": np.asarray(bq, np.float32),
        "bk": np.asarray(bk, np.float32),
        "bv": np.asarray(bv, np.float32),
        "bo": np.asarray(bo, np.float32),
    }
    in_maps = []
    for b in range(B):
        m = dict(shared)
        m["x32"] = xf[b]
        m["x16"] = xf[b].astype(_BF)
        in_maps.append(m)
    return in_maps


def kernel(x, Wq, bq, Wk, bk, Wv, bv, Wo, bo, _trace=False):
    from concourse.bass_utils import run_bass_kernel_spmd

    x = np.asarray(x)
    B, C, H, W = x.shape
    in_maps = _prep_maps(x, Wq, bq, Wk, bk, Wv, bv, Wo, bo)
    nc = build_nc()
    res = run_bass_kernel_spmd(nc, in_maps, core_ids=list(range(B)), trace=_trace)
    out = np.stack([res.results[b]["out"] for b in range(B)])
    out = out.reshape(B, C, H, W).astype(np.float32)
    if _trace:
        kernel.last_results = res
    return out



# revision 8
# speedup vs baseline: 1.2342x; 1.2342x over previous
"""Multi-head attention (dense_transformer) Trainium2 Bass kernel.

Problem: x[8, 512, 32, 32]; per-batch 1x1-conv QKV projections, 8-head
attention over N=H*W=1024 positions (head_dim 64), output projection,
residual. Sharding: data-parallel over batch B=8 across the 8 cores --
one batch element per core, no collectives.

Per-core dataflow (all matmuls fp8e4; DoubleRow perf mode where the
contraction allows 2x128 planes -> 0.5 cyc/row):
  - Host pre-scales Q/K/V/O weight paths by 8 so fp8e4m3 stays out of
    subnormals; the net x64 factor on the output projection is divided
    back out in the residual add (scalar_tensor_tensor). K bias is
    dropped (softmax-invariant), V bias is folded into the residual
    (x32b = x + Wo@bv + bo), Q bias rides the ACT cast for free.
  - Q/K projections: DoubleRow over c=(t,s,p) planes; PSUM->SBUF cast
    to fp8 on ACT (Q: Identity+bias, K: Copy).
  - V projection: out is [j, o]-transposed (lhsT=x8), cast into VT
    tiles laid out [128 j, 2 jt-plane, 8 h, 64 d + 64 ones] so the AV
    matmul gets per-head data and a 64-wide ones block in one lhsT.
  - S^T[j,i] = K_h^T Q_h per head, plain fp8 (contraction 64).
  - exp: split ACT (native Exp -> fp8, scale 1/512) / DVE (Schraudolph
    int8 bit-trick: bits = S*a+b -> int8, bitcast to fp8e4m3).
  - AV: DoubleRow over jt-pair planes; lhsT cols = [64 V | 64 ones] so
    PSUM rows 0-63 = raw head output, rows 64-127 = softmax denominator
    broadcast across partitions. Normalize = one DVE divide -> fp8 O8.
  - Output projection: DoubleRow over (g,s) channel planes; residual =
    scalar_tensor_tensor((psum * 1/64) + x32b) -> fp32 out DMA.
"""

import sys

if "/opt/trn_rl_repo" not in sys.path:
    sys.path.insert(0, "/opt/trn_rl_repo")

import numpy as np
import ml_dtypes

import concourse.bass as bass
import concourse.mybir as mybir
from concourse.tile import TileContext

DIM = 512
NH = 8
HD = 64
N = 1024
P = 128
F32 = mybir.dt.float32
FP8 = mybir.dt.float8e4
I8 = mybir.dt.int8
AOP = mybir.AluOpType
EXP = mybir.ActivationFunctionType.Exp
IDENT = mybir.ActivationFunctionType.Identity
COPY = mybir.ActivationFunctionType.Copy
DR = mybir.MatmulPerfMode.DoubleRow

# Schraudolph exp on fp8e4m3 bits: for y=e^(S/512), bits = 8*(S/512*log2e
# + 7) + sigma.  a = 8*log2(e)/512; b = 56 + sigma - 0.5-ish; tuned for
# truncation-style float->int casts.
SCH_A = 8.0 * 1.4426950408889634 / 512.0
SCH_B = 56.0

# exp engine assignment: 64 (head, jt) tiles; True -> ACT, False -> DVE.
# ~39 on ACT, 25 on DVE balances ACT (casts+exp) against DVE
# (exp+divide+resid).
def _default_exp_on_act(idx):
    return (idx * 39) % 64 < 39


class FixedTileContext(TileContext):
    """Works around a walrus/bass snapshot mismatch: this walrus build
    accepts only one sync-wait command per instruction, but Tile's wait
    assigner happily attaches several. After scheduling, excess waits on
    any instruction are peeled off onto same-engine NOPs inserted right
    before it (same blocking semantics: the engine executes in order)."""

    MAX_WAITS = 1
    MAX_WAITS_DATA = 1
    _wsplit_ctr = 0

    def _split_sync_waits(self):
        seq_only = mybir.SEQUENCER_ONLY_OPCODES
        for fn in self.nc.m.functions:
            for blk in fn.blocks:
                insts = list(blk.instructions)
                out = []
                for inst in insts:
                    si = inst.sync_info
                    limit = (
                        self.MAX_WAITS
                        if inst.opcode in seq_only
                        else self.MAX_WAITS_DATA
                    )
                    if si is not None and len(si.on_wait) > limit:
                        waits = list(si.on_wait)
                        movers = waits[:-limit]
                        keep = waits[-limit:]
                        del si.on_wait[:]
                        for w in keep:
                            si.on_wait.append(w)
                        for w in movers:
                            FixedTileContext._wsplit_ctr += 1
                            nop = mybir.InstNoOp(
                                name=f"wsplit-{FixedTileContext._wsplit_ctr}",
                                ins=[],
                                outs=[],
                            )
                            nop.engine = inst.engine
                            nop.sync_info = mybir.SyncInfo(on_wait=[w], on_update=[])
                            out.append(nop)
                    out.append(inst)
                if len(out) != len(insts):
                    del blk.instructions[:]
                    for i in out:
                        blk.add_instruction(i)

    split_on_exit = True

    def __exit__(self, *exc):
        ret = super().__exit__(*exc)
        if exc[0] is None and self.split_on_exit:
            self._split_sync_waits()
        return ret


def build_nc(split_waits=True, exp_on_act=_default_exp_on_act):
    nc = bass.Bass()

    # c-plane layout for DoubleRow contractions: c = 128*(2t+s) + p
    x8d = nc.dram_tensor("x8", [2, P, 2, N], FP8, kind="ExternalInput")
    wq8d = nc.dram_tensor("wq8", [2, P, 2, DIM], FP8, kind="ExternalInput")
    wk8d = nc.dram_tensor("wk8", [2, P, 2, DIM], FP8, kind="ExternalInput")
    wv8d = nc.dram_tensor("wv8", [2, P, 2, DIM], FP8, kind="ExternalInput")
    wo8d = nc.dram_tensor("wo8", [2, P, 2, DIM], FP8, kind="ExternalInput")
    bqd = nc.dram_tensor("bqp", [P, 4], F32, kind="ExternalInput")
    x32d = nc.dram_tensor("x32b", [DIM, N], F32, kind="ExternalInput")
    outd = nc.dram_tensor("out", [DIM, N], F32, kind="ExternalOutput")

    FixedTileContext.split_on_exit = split_waits
    with FixedTileContext(nc) as tc:
        with (
            tc.tile_pool(name="persist", bufs=1) as persist,
            tc.tile_pool(name="ostage", bufs=4) as ostage,
            tc.tile_pool(name="psS", bufs=2, space="PSUM") as psS,
            tc.tile_pool(name="psAV", bufs=2, space="PSUM") as psAV,
        ):
            # ---------------- input loads (all on SP queue) ----------------
            def load(dram_ap, shape, dt, name):
                t = persist.tile(shape, dt, tag=name, name=name)
                nc.sync.dma_start(out=t, in_=dram_ap)
                return t

            x8 = [load(x8d[t], [P, 2, N], FP8, f"x8_{t}") for t in range(2)]
            wq8 = [load(wq8d[t], [P, 2, DIM], FP8, f"wq8_{t}") for t in range(2)]
            wk8 = [load(wk8d[t], [P, 2, DIM], FP8, f"wk8_{t}") for t in range(2)]
            bq_sb = load(bqd[:], [P, 4], F32, "bq")
            wv8 = [load(wv8d[t], [P, 2, DIM], FP8, f"wv8_{t}") for t in range(2)]
            wo8 = [load(wo8d[g], [P, 2, DIM], FP8, f"wo8_{g}") for g in range(2)]
            x32 = [
                load(
                    x32d.rearrange("(t p) n -> t p n", p=P)[t],
                    [P, N],
                    F32,
                    f"x32_{t}",
                )
                for t in range(4)
            ]

            # VT tiles: [128 j, 2 jt-plane, 8 h, 64 d + 64 ones] per jt-pair.
            # ones blocks memset once on gpsimd (otherwise idle).
            vt = []
            for jp in range(4):
                t = persist.tile([P, 2, NH, P], FP8, tag=f"vt{jp}", name=f"vt{jp}")
                nc.gpsimd.memset(t[:, :, :, HD:P], 1.0)
                vt.append(t)

            # Q/K fp8 tiles, [128 (2 heads x 64 d), 1024] per ot
            q8 = [
                persist.tile([P, N], FP8, tag=f"q8_{o}", name=f"q8_{o}")
                for o in range(4)
            ]
            k8 = [
                persist.tile([P, N], FP8, tag=f"k8_{o}", name=f"k8_{o}")
                for o in range(4)
            ]
            # P8 tiles: [128 j, 2 jt-plane, 1024 i] per (head, jt-pair)
            p8 = [
                [
                    persist.tile([P, 2, N], FP8, tag=f"p8_{h}_{jp}", name=f"p8_{h}_{jp}")
                    for jp in range(4)
                ]
                for h in range(NH)
            ]
            # O8: [128 p, 2 s, 1024] per g; att-channel c' = 128*(2g+s)+p
            o8 = [
                persist.tile([P, 2, N], FP8, tag=f"o8_{g}", name=f"o8_{g}")
                for g in range(2)
            ]

            exp_idx = [0]

            pools = {}

            def proj_qk(ot):
                # Q then K for o-block ot; casts on ACT
                for which, w8, dst in (("q", wq8, q8), ("k", wk8, k8)):
                    for nh2 in range(2):
                        ps = pools["proj"].tile(
                            [P, DIM], F32, tag="pp", name=f"pp{which}{ot}{nh2}"
                        )
                        for t in range(2):
                            nc.tensor.matmul(
                                ps,
                                lhsT=w8[t][:, :, ot * P : (ot + 1) * P],
                                rhs=x8[t][:, :, nh2 * DIM : (nh2 + 1) * DIM],
                                start=(t == 0),
                                stop=(t == 1),
                                perf_mode=DR,
                            )
                        sl = slice(nh2 * DIM, (nh2 + 1) * DIM)
                        if which == "q":
                            nc.scalar.activation(
                                dst[ot][:, sl], ps, IDENT, bias=bq_sb[:, ot : ot + 1]
                            )
                        else:
                            nc.scalar.activation(dst[ot][:, sl], ps, COPY)

            def proj_v(jt):
                ps = pools["proj"].tile([P, DIM], F32, tag="pp", name=f"ppv{jt}")
                for t in range(2):
                    nc.tensor.matmul(
                        ps,
                        lhsT=x8[t][:, :, jt * P : (jt + 1) * P],
                        rhs=wv8[t],
                        start=(t == 0),
                        stop=(t == 1),
                        perf_mode=DR,
                    )
                # cast into vt[jt//2] plane jt%2, head-major data cols
                dst = vt[jt // 2][:, jt % 2, :, 0:HD]
                nc.scalar.activation(dst, ps.rearrange("p (h d) -> p h d", h=NH), COPY)

            def s_head(h):
                # S^T tiles + exp for one head: 8 jt, each [128 j, 1024 i]
                ot, half = h // 2, h % 2
                base = half * HD
                for jt in range(8):
                    ps = psS.tile([P, N], F32, tag="ps", name=f"ps{h}_{jt}")
                    for ih in range(2):
                        isl = slice(ih * DIM, (ih + 1) * DIM)
                        nc.tensor.matmul(
                            ps[:, isl],
                            lhsT=k8[ot][base : base + HD, jt * P : (jt + 1) * P],
                            rhs=q8[ot][base : base + HD, isl],
                            start=True,
                            stop=True,
                        )
                    dst = p8[h][jt // 2][:, jt % 2, :]
                    if exp_on_act(exp_idx[0]):
                        nc.scalar.activation(dst, ps, EXP, scale=1.0 / 512.0)
                    else:
                        nc.vector.tensor_scalar(
                            dst.bitcast(I8), ps, SCH_A, SCH_B, AOP.mult, AOP.add
                        )
                    exp_idx[0] += 1

            def av_head(h):
                # AV + denominator in one matmul per (ih, jt-pair); then one
                # divide per ih -> fp8 O8
                g, s, prow = h // 4, (h // 2) % 2, (h % 2) * HD
                for ih in range(2):
                    po = psAV.tile([P, DIM], F32, tag="po", name=f"po{h}_{ih}")
                    isl = slice(ih * DIM, (ih + 1) * DIM)
                    for jp in range(4):
                        nc.tensor.matmul(
                            po,
                            lhsT=vt[jp][:, :, h, :],
                            rhs=p8[h][jp][:, :, isl],
                            start=(jp == 0),
                            stop=(jp == 3),
                            perf_mode=DR,
                        )
                    nc.vector.tensor_tensor(
                        o8[g][prow : prow + HD, s, isl],
                        po[0:HD, :],
                        po[HD:P, :],
                        AOP.divide,
                    )

            def out_block(ot, nh2):
                ps = pools["out"].tile([P, DIM], F32, tag="pso", name=f"pso{ot}{nh2}")
                isl = slice(nh2 * DIM, (nh2 + 1) * DIM)
                for g in range(2):
                    nc.tensor.matmul(
                        ps,
                        lhsT=wo8[g][:, :, ot * P : (ot + 1) * P],
                        rhs=o8[g][:, :, isl],
                        start=(g == 0),
                        stop=(g == 1),
                        perf_mode=DR,
                    )
                ob = ostage.tile([P, DIM], F32, tag="ob", name="ob")
                nc.vector.scalar_tensor_tensor(
                    ob, ps, 1.0 / 64.0, x32[ot][:, isl], AOP.mult, AOP.add
                )
                nc.sync.dma_start(
                    out=outd.rearrange("(t p) n -> t p n", p=P)[ot][:, isl], in_=ob
                )

            # ---------------- schedule ----------------
            # psProj (2 banks) closes before psO (2 banks) opens; psS (4) +
            # psAV (2) live throughout: peak 8 banks.
            with tc.tile_pool(name="psProj", bufs=2, space="PSUM") as psProj:
                pools["proj"] = psProj
                proj_qk(0)
                for jt in range(4):
                    proj_v(jt)
                s_head(0)
                for jt in range(4, 8):
                    proj_v(jt)
                s_head(1)
                proj_qk(1)
                s_head(2)
                av_head(0)
                s_head(3)
                av_head(1)
                proj_qk(2)
                s_head(4)
                av_head(2)
                s_head(5)
                av_head(3)
                proj_qk(3)
            with tc.tile_pool(name="psO", bufs=2, space="PSUM") as psO:
                pools["out"] = psO
                s_head(6)
                av_head(4)
                s_head(7)
                av_head(5)
                av_head(6)
                av_head(7)
                for ot in range(4):
                    for nh2 in range(2):
                        out_block(ot, nh2)
    return nc


_F8 = ml_dtypes.float8_e4m3


def _plane(a):
    # [c, m] -> [2 t, 128 p, 2 s, m] with c = 128*(2t+s)+p
    m = a.shape[1]
    return np.ascontiguousarray(
        a.reshape(2, 2, P, m).transpose(0, 2, 1, 3)
    )


def _prep_maps(x, Wq, bq, Wk, bk, Wv, bv, Wo, bo):
    # plain numpy up front: inputs may arrive as jax device arrays and
    # transforming those would trigger on-device jax execution
    x, Wq, bq, Wk, bk, Wv, bv, Wo, bo = (
        np.asarray(a, dtype=np.float32) if np.asarray(a).dtype != np.float32
        else np.asarray(a)
        for a in (x, Wq, bq, Wk, bk, Wv, bv, Wo, bo)
    )
    B, C, H, W = x.shape
    xf = np.ascontiguousarray(x.reshape(B, C, H * W)).astype(np.float32)
    rb = (Wo @ bv + bo).astype(np.float32)  # V-bias folded through Wo
    shared = {
        "wq8": _plane(8.0 * Wq.T).astype(_F8),
        "wk8": _plane(8.0 * Wk.T).astype(_F8),
        "wv8": _plane(8.0 * Wv.T).astype(_F8),
        "wo8": _plane(8.0 * Wo.T).astype(_F8),
        "bqp": np.ascontiguousarray((8.0 * bq).reshape(4, P).T).astype(np.float32),
    }
    in_maps = []
    for b in range(B):
        m = dict(shared)
        m["x8"] = _plane(xf[b]).astype(_F8)
        m["x32b"] = xf[b] + rb[:, None]
        in_maps.append(m)
    return in_maps


def kernel(x, Wq, bq, Wk, bk, Wv, bv, Wo, bo, _trace=False):
    from concourse.bass_utils import run_bass_kernel_spmd

    x = np.asarray(x)
    B, C, H, W = x.shape
    in_maps = _prep_maps(x, Wq, bq, Wk, bk, Wv, bv, Wo, bo)
    nc = build_nc()
    res = run_bass_kernel_spmd(nc, in_maps, core_ids=list(range(B)), trace=_trace)
    out = np.stack([res.results[b]["out"] for b in range(B)])
    out = out.reshape(B, C, H, W).astype(np.float32)
    if _trace:
        kernel.last_results = res
    return out


# revision 15
# speedup vs baseline: 1.2500x; 1.0128x over previous
"""Multi-head attention (dense_transformer) Trainium2 Bass kernel.

Problem: x[8, 512, 32, 32]; per-batch 1x1-conv QKV projections, 8-head
attention over N=H*W=1024 positions (head_dim 64), output projection,
residual. Sharding: data-parallel over batch B=8 across the 8 cores --
one batch element per core, no collectives.

Per-core dataflow (all matmuls fp8e4; DoubleRow perf mode where the
contraction allows 2x128 planes -> 0.5 cyc/row):
  - Host pre-scales Q/K/V/O weight paths by 8 so fp8e4m3 stays out of
    subnormals; the net x64 factor on the output projection is divided
    back out in the residual add (scalar_tensor_tensor). K bias is
    dropped (softmax-invariant), V bias is folded into the residual
    (x32b = x + Wo@bv + bo), Q bias rides the ACT cast for free.
  - Q/K projections: DoubleRow over c=(t,s,p) planes; PSUM->SBUF cast
    to fp8 on ACT (Q: Identity+bias, K: Copy).
  - V projection: out is [j, o]-transposed (lhsT=x8), cast into VT
    tiles laid out [128 j, 2 jt-plane, 8 h, 64 d + 64 ones] so the AV
    matmul gets per-head data and a 64-wide ones block in one lhsT.
  - S^T[j,i] = K_h^T Q_h per head, plain fp8 (contraction 64).
  - exp: split ACT (native Exp -> fp8, scale 1/512) / DVE (Schraudolph
    int8 bit-trick: bits = S*a+b -> int8, bitcast to fp8e4m3).
  - AV: DoubleRow over jt-pair planes; lhsT cols = [64 V | 64 ones] so
    PSUM rows 0-63 = raw head output, rows 64-127 = softmax denominator
    broadcast across partitions. Normalize = one DVE divide -> fp8 O8.
  - Output projection: DoubleRow over (g,s) channel planes; residual =
    scalar_tensor_tensor((psum * 1/64) + x32b) -> fp32 out DMA.
"""

import sys

if "/opt/trn_rl_repo" not in sys.path:
    sys.path.insert(0, "/opt/trn_rl_repo")

import numpy as np
import ml_dtypes

import concourse.bass as bass
import concourse.mybir as mybir
from concourse.tile import TileContext

DIM = 512
NH = 8
HD = 64
N = 1024
P = 128
F32 = mybir.dt.float32
FP8 = mybir.dt.float8e4
I8 = mybir.dt.int8
AOP = mybir.AluOpType
EXP = mybir.ActivationFunctionType.Exp
IDENT = mybir.ActivationFunctionType.Identity
COPY = mybir.ActivationFunctionType.Copy
DR = mybir.MatmulPerfMode.DoubleRow

# Schraudolph exp on fp8e4m3 bits: for y=e^(S/512), bits = 8*(S/512*log2e
# + 7) + sigma.  a = 8*log2(e)/512; b = 56 + sigma - 0.5-ish; tuned for
# truncation-style float->int casts.
SCH_A = 8.0 * 1.4426950408889634 / 512.0
SCH_B = 56.0

# exp engine assignment: 64 (head, jt) tiles spread across ACT (native
# Exp), DVE (Schraudolph tensor_scalar) and Pool (Schraudolph from a
# DMA-staged SBUF copy of the PSUM tile -- gpsimd has no PSUM port).
def _mk_exp_pattern(na, nd, np_):
    quota = {"A": na, "D": nd, "P": np_}
    total = na + nd + np_
    credit = {"A": 0, "D": 0, "P": 0}
    out = []
    for i in range(total):
        e = max("ADP", key=lambda k: quota[k] / total * (i + 1) - credit[k])
        out.append(e)
        credit[e] += 1
    return out


# gpsimd has no PSUM port and bass DMA moves only SBUF/DRAM, so nothing
# can stage S tiles into SBUF for Pool without paying the same ACT/DVE
# read anyway: exp is a strict ACT/DVE split.
_DEFAULT_EXP_PATTERN = _mk_exp_pattern(39, 25, 0)


class FixedTileContext(TileContext):
    """Works around a walrus/bass snapshot mismatch: this walrus build
    accepts only one sync-wait command per instruction, but Tile's wait
    assigner happily attaches several. After scheduling, excess waits on
    any instruction are peeled off onto same-engine NOPs inserted right
    before it (same blocking semantics: the engine executes in order)."""

    MAX_WAITS = 1
    MAX_WAITS_DATA = 1
    _wsplit_ctr = 0

    def _split_sync_waits(self):
        seq_only = mybir.SEQUENCER_ONLY_OPCODES
        for fn in self.nc.m.functions:
            for blk in fn.blocks:
                insts = list(blk.instructions)
                out = []
                for inst in insts:
                    si = inst.sync_info
                    limit = (
                        self.MAX_WAITS
                        if inst.opcode in seq_only
                        else self.MAX_WAITS_DATA
                    )
                    if si is not None and len(si.on_wait) > limit:
                        waits = list(si.on_wait)
                        movers = waits[:-limit]
                        keep = waits[-limit:]
                        del si.on_wait[:]
                        for w in keep:
                            si.on_wait.append(w)
                        for w in movers:
                            FixedTileContext._wsplit_ctr += 1
                            nop = mybir.InstNoOp(
                                name=f"wsplit-{FixedTileContext._wsplit_ctr}",
                                ins=[],
                                outs=[],
                            )
                            nop.engine = inst.engine
                            nop.sync_info = mybir.SyncInfo(on_wait=[w], on_update=[])
                            out.append(nop)
                    out.append(inst)
                if len(out) != len(insts):
                    del blk.instructions[:]
                    for i in out:
                        blk.add_instruction(i)

    split_on_exit = True

    def __exit__(self, *exc):
        ret = super().__exit__(*exc)
        if exc[0] is None and self.split_on_exit:
            self._split_sync_waits()
        return ret


def build_nc(split_waits=True, exp_pattern=None):
    if exp_pattern is None:
        exp_pattern = _DEFAULT_EXP_PATTERN
    nc = bass.Bass()

    # c-plane layout for DoubleRow contractions: c = 128*(2t+s) + p
    x8d = nc.dram_tensor("x8", [2, P, 2, N], FP8, kind="ExternalInput")
    wq8d = nc.dram_tensor("wq8", [2, P, 2, DIM], FP8, kind="ExternalInput")
    wk8d = nc.dram_tensor("wk8", [2, P, 2, DIM], FP8, kind="ExternalInput")
    wv8d = nc.dram_tensor("wv8", [2, P, 2, DIM], FP8, kind="ExternalInput")
    wo8d = nc.dram_tensor("wo8", [2, P, 2, DIM], FP8, kind="ExternalInput")
    bqd = nc.dram_tensor("bqp", [P, 4], F32, kind="ExternalInput")
    x32d = nc.dram_tensor("x32b", [DIM, N], F32, kind="ExternalInput")
    outd = nc.dram_tensor("out", [DIM, N], F32, kind="ExternalOutput")

    FixedTileContext.split_on_exit = split_waits
    with FixedTileContext(nc) as tc:
        with (
            tc.tile_pool(name="persist", bufs=1) as persist,
            tc.tile_pool(name="ostage", bufs=4) as ostage,
            tc.tile_pool(name="stage", bufs=3) as stage,
            tc.tile_pool(name="psS", bufs=2, space="PSUM") as psS,
            tc.tile_pool(name="psAV", bufs=2, space="PSUM") as psAV,
        ):
            # ---------------- input loads (all on SP queue) ----------------
            def load(dram_ap, shape, dt, name):
                t = persist.tile(shape, dt, tag=name, name=name)
                nc.sync.dma_start(out=t, in_=dram_ap)
                return t

            bq_sb = load(bqd[:], [P, 4], F32, "bq")
            x8, wq8, wk8 = [], [], []
            for t in range(2):
                x8.append(load(x8d[t], [P, 2, N], FP8, f"x8_{t}"))
                wq8.append(load(wq8d[t], [P, 2, DIM], FP8, f"wq8_{t}"))
                wk8.append(load(wk8d[t], [P, 2, DIM], FP8, f"wk8_{t}"))
            wv8 = [load(wv8d[t], [P, 2, DIM], FP8, f"wv8_{t}") for t in range(2)]
            wo8 = [load(wo8d[g], [P, 2, DIM], FP8, f"wo8_{g}") for g in range(2)]
            x32 = [
                load(
                    x32d.rearrange("(t p) n -> t p n", p=P)[t],
                    [P, N],
                    F32,
                    f"x32_{t}",
                )
                for t in range(4)
            ]

            # VT tiles: [128 j, 2 jt-plane, 8 h, 64 d + 64 ones] per jt-pair.
            # ones blocks memset once on gpsimd (otherwise idle).
            vt = []
            for jp in range(4):
                t = persist.tile([P, 2, NH, P], FP8, tag=f"vt{jp}", name=f"vt{jp}")
                nc.gpsimd.memset(t[:, :, :, HD:P], 1.0)
                vt.append(t)

            # Q/K fp8 tiles, [128 (2 heads x 64 d), 1024] per ot
            q8 = [
                persist.tile([P, N], FP8, tag=f"q8_{o}", name=f"q8_{o}")
                for o in range(4)
            ]
            k8 = [
                persist.tile([P, N], FP8, tag=f"k8_{o}", name=f"k8_{o}")
                for o in range(4)
            ]
            # P8 tiles: [128 j, 2 jt-plane, 1024 i] per (head, jt-pair)
            p8 = [
                [
                    persist.tile([P, 2, N], FP8, tag=f"p8_{h}_{jp}", name=f"p8_{h}_{jp}")
                    for jp in range(4)
                ]
                for h in range(NH)
            ]
            # O8: [128 p, 2 s, 1024] per g; att-channel c' = 128*(2g+s)+p
            o8 = [
                persist.tile([P, 2, N], FP8, tag=f"o8_{g}", name=f"o8_{g}")
                for g in range(2)
            ]

            exp_idx = [0]

            pools = {}

            def proj_qk(ot):
                # Q then K for o-block ot; casts on ACT
                for which, w8, dst in (("q", wq8, q8), ("k", wk8, k8)):
                    for nh2 in range(2):
                        ps = pools["proj"].tile(
                            [P, DIM], F32, tag="pp", name=f"pp{which}{ot}{nh2}"
                        )
                        for t in range(2):
                            nc.tensor.matmul(
                                ps,
                                lhsT=w8[t][:, :, ot * P : (ot + 1) * P],
                                rhs=x8[t][:, :, nh2 * DIM : (nh2 + 1) * DIM],
                                start=(t == 0),
                                stop=(t == 1),
                                perf_mode=DR,
                            )
                        sl = slice(nh2 * DIM, (nh2 + 1) * DIM)
                        if which == "q":
                            nc.scalar.activation(
                                dst[ot][:, sl], ps, IDENT, bias=bq_sb[:, ot : ot + 1]
                            )
                        else:
                            nc.scalar.activation(dst[ot][:, sl], ps, COPY)

            def proj_v(jt):
                ps = pools["proj"].tile([P, DIM], F32, tag="pp", name=f"ppv{jt}")
                for t in range(2):
                    nc.tensor.matmul(
                        ps,
                        lhsT=x8[t][:, :, jt * P : (jt + 1) * P],
                        rhs=wv8[t],
                        start=(t == 0),
                        stop=(t == 1),
                        perf_mode=DR,
                    )
                # cast into vt[jt//2] plane jt%2, head-major data cols
                dst = vt[jt // 2][:, jt % 2, :, 0:HD]
                nc.vector.tensor_copy(dst, ps.rearrange("p (h d) -> p h d", h=NH))

            def s_head(h):
                # S^T tiles + exp for one head: 8 jt, each [128 j, 1024 i]
                ot, half = h // 2, h % 2
                base = half * HD
                for jt in range(8):
                    ps = psS.tile([P, N], F32, tag="ps", name=f"ps{h}_{jt}")
                    for ih in range(2):
                        isl = slice(ih * DIM, (ih + 1) * DIM)
                        nc.tensor.matmul(
                            ps[:, isl],
                            lhsT=k8[ot][base : base + HD, jt * P : (jt + 1) * P],
                            rhs=q8[ot][base : base + HD, isl],
                            start=True,
                            stop=True,
                        )
                    dst = p8[h][jt // 2][:, jt % 2, :]
                    eng = exp_pattern[exp_idx[0]]
                    if eng == "A":
                        nc.scalar.activation(dst, ps, EXP, scale=1.0 / 512.0)
                    elif eng == "D":
                        nc.vector.tensor_scalar(
                            dst.bitcast(I8), ps, SCH_A, SCH_B, AOP.mult, AOP.add
                        )
                    else:
                        st = stage.tile([P, N], F32, tag="st", name=f"st{h}_{jt}")
                        nc.sync.dma_start(out=st, in_=ps)
                        nc.gpsimd.tensor_scalar(
                            dst.bitcast(I8), st, SCH_A, SCH_B, AOP.mult, AOP.add
                        )
                    exp_idx[0] += 1

            def av_head(h):
                # AV + denominator in one matmul per (ih, jt-pair); then one
                # divide per ih -> fp8 O8
                g, s, prow = h // 4, (h // 2) % 2, (h % 2) * HD
                for ih in range(2):
                    po = psAV.tile([P, DIM], F32, tag="po", name=f"po{h}_{ih}")
                    isl = slice(ih * DIM, (ih + 1) * DIM)
                    for jp in range(4):
                        nc.tensor.matmul(
                            po,
                            lhsT=vt[jp][:, :, h, :],
                            rhs=p8[h][jp][:, :, isl],
                            start=(jp == 0),
                            stop=(jp == 3),
                            perf_mode=DR,
                        )
                    nc.vector.tensor_tensor(
                        o8[g][prow : prow + HD, s, isl],
                        po[0:HD, :],
                        po[HD:P, :],
                        AOP.divide,
                    )

            def out_block(ot, nh2):
                ps = pools["out"].tile([P, DIM], F32, tag="pso", name=f"pso{ot}{nh2}")
                isl = slice(nh2 * DIM, (nh2 + 1) * DIM)
                for g in range(2):
                    nc.tensor.matmul(
                        ps,
                        lhsT=wo8[g][:, :, ot * P : (ot + 1) * P],
                        rhs=o8[g][:, :, isl],
                        start=(g == 0),
                        stop=(g == 1),
                        perf_mode=DR,
                    )
                ob = ostage.tile([P, DIM], F32, tag="ob", name="ob")
                nc.vector.scalar_tensor_tensor(
                    ob, ps, 1.0 / 64.0, x32[ot][:, isl], AOP.mult, AOP.add
                )
                nc.sync.dma_start(
                    out=outd.rearrange("(t p) n -> t p n", p=P)[ot][:, isl], in_=ob
                )

            # ---------------- schedule ----------------
            # psProj (2 banks) closes before psO (2 banks) opens; psS (4) +
            # psAV (2) live throughout: peak 8 banks.
            with tc.tile_pool(name="psProj", bufs=2, space="PSUM") as psProj:
                pools["proj"] = psProj
                proj_qk(0)
                for jt in range(4):
                    proj_v(jt)
                s_head(0)
                for jt in range(4, 8):
                    proj_v(jt)
                s_head(1)
                proj_qk(1)
                s_head(2)
                av_head(0)
                s_head(3)
                av_head(1)
                proj_qk(2)
                s_head(4)
                av_head(2)
                s_head(5)
                av_head(3)
                proj_qk(3)
            with tc.tile_pool(name="psO", bufs=2, space="PSUM") as psO:
                pools["out"] = psO
                s_head(6)
                av_head(4)
                s_head(7)
                av_head(5)
                av_head(6)
                av_head(7)
                for ot in range(4):
                    for nh2 in range(2):
                        out_block(ot, nh2)
    return nc


_F8 = ml_dtypes.float8_e4m3


def _plane(a):
    # [c, m] -> [2 t, 128 p, 2 s, m] with c = 128*(2t+s)+p
    m = a.shape[1]
    return np.ascontiguousarray(
        a.reshape(2, 2, P, m).transpose(0, 2, 1, 3)
    )


def _prep_maps(x, Wq, bq, Wk, bk, Wv, bv, Wo, bo):
    # plain numpy up front: inputs may arrive as jax device arrays and
    # transforming those would trigger on-device jax execution
    x, Wq, bq, Wk, bk, Wv, bv, Wo, bo = (
        np.asarray(a, dtype=np.float32) if np.asarray(a).dtype != np.float32
        else np.asarray(a)
        for a in (x, Wq, bq, Wk, bk, Wv, bv, Wo, bo)
    )
    B, C, H, W = x.shape
    xf = np.ascontiguousarray(x.reshape(B, C, H * W)).astype(np.float32)
    rb = (Wo @ bv + bo).astype(np.float32)  # V-bias folded through Wo
    shared = {
        "wq8": _plane(8.0 * Wq.T).astype(_F8),
        "wk8": _plane(8.0 * Wk.T).astype(_F8),
        "wv8": _plane(8.0 * Wv.T).astype(_F8),
        "wo8": _plane(8.0 * Wo.T).astype(_F8),
        "bqp": np.ascontiguousarray((8.0 * bq).reshape(4, P).T).astype(np.float32),
    }
    in_maps = []
    for b in range(B):
        m = dict(shared)
        m["x8"] = _plane(xf[b]).astype(_F8)
        m["x32b"] = xf[b] + rb[:, None]
        in_maps.append(m)
    return in_maps


def kernel(x, Wq, bq, Wk, bk, Wv, bv, Wo, bo, _trace=False):
    from concourse.bass_utils import run_bass_kernel_spmd

    x = np.asarray(x)
    B, C, H, W = x.shape
    in_maps = _prep_maps(x, Wq, bq, Wk, bk, Wv, bv, Wo, bo)
    nc = build_nc()
    res = run_bass_kernel_spmd(nc, in_maps, core_ids=list(range(B)), trace=_trace)
    out = np.stack([res.results[b]["out"] for b in range(B)])
    out = out.reshape(B, C, H, W).astype(np.float32)
    if _trace:
        kernel.last_results = res
    return out
